# revision 2
# baseline (speedup 1.0000x reference)
"""Fused dequant + residual-add + RMSNorm + int8 requant for TRN2 (8 NeuronCores).

Sharding: tokens (rows) split evenly across the 8 cores; the hidden-dim
reduction stays local and `weight` is replicated.

Per core (2048 x 4096), per 128-row tile:
  ACT : xf = x_i32 * a                     (dequant, int32 -> f32)
  DVE : res_new = residual + xf            (tensor_tensor add, in place)
  DVE : bn_stats/bn_aggr -> mean, var      (mean(res^2) = var + mean^2)
  ACT : rms = sqrt(ms + eps);  DVE: rstd = 1/rms
  DVE : yw = res_new * w                   (weight broadcast over partitions)
  ACT : out_i8 = yw * rstd                 (f32 -> int8 is RNE + saturating)

The f32->int8 output conversion on ACT/DVE was verified on hardware to match
np.clip(np.round(x), -128, 127) exactly, so no explicit rounding pass is needed.
"""

import os

import numpy as np

import concourse.bacc as bacc
import concourse.bass as bass
import concourse.tile as tile
from concourse import mybir
from concourse.bass_utils import run_bass_kernel_spmd

TOKENS = 16384
HIDDEN = 4096
N_CORES = 8
ROWS = TOKENS // N_CORES  # 2048 rows per core
P = 128                   # SBUF partitions
NT = ROWS // P            # 16 row-tiles per core
EPS = 1e-6
BN_CHUNK = 512            # bn_stats free-dim hardware limit

_cache: dict = {}
last_results = None  # BassKernelResults of the most recent run (for profiling)


def _build(a: float):
    nc = bacc.Bacc(
        "TRN2", target_bir_lowering=False, debug=False, num_devices=N_CORES
    )
    residual = nc.dram_tensor(
        "residual", [ROWS, HIDDEN], mybir.dt.float32, kind="ExternalInput"
    ).ap()
    x = nc.dram_tensor(
        "x", [ROWS, HIDDEN], mybir.dt.int32, kind="ExternalInput"
    ).ap()
    weight = nc.dram_tensor(
        "weight", [HIDDEN], mybir.dt.float32, kind="ExternalInput"
    ).ap()
    res_new = nc.dram_tensor(
        "res_new", [ROWS, HIDDEN], mybir.dt.float32, kind="ExternalOutput"
    ).ap()
    out_i8 = nc.dram_tensor(
        "out_i8", [ROWS, HIDDEN], mybir.dt.int8, kind="ExternalOutput"
    ).ap()

    with tile.TileContext(nc) as tc:
        with (
            tc.tile_pool(name="singles", bufs=1) as singles,
            tc.tile_pool(name="work", bufs=3) as work,
            tc.tile_pool(name="stats", bufs=4) as stats_pool,
        ):
            # weight broadcast to all 128 partitions (one-time, 2 MiB)
            w_b = singles.tile([P, HIDDEN], mybir.dt.float32)
            w_bcast = bass.AP(
                tensor=weight.tensor,
                offset=weight.offset,
                ap=[[0, P], list(weight.ap[0])],
            )
            nc.gpsimd.dma_start(out=w_b[:], in_=w_bcast)
            eps_t = singles.tile([P, 1], mybir.dt.float32)
            nc.vector.memset(eps_t[:], EPS)

            for it in range(NT):
                r0 = it * P
                xi = work.tile([P, HIDDEN], mybir.dt.int32, tag="xi")
                nc.sync.dma_start(out=xi[:], in_=x[r0 : r0 + P, :])
                res = work.tile([P, HIDDEN], mybir.dt.float32, tag="res")
                nc.sync.dma_start(out=res[:], in_=residual[r0 : r0 + P, :])

                xf = work.tile([P, HIDDEN], mybir.dt.float32, tag="xf")
                nc.scalar.mul(xf[:], xi[:], a)
                nc.vector.tensor_add(res[:], res[:], xf[:])
                nc.sync.dma_start(out=res_new[r0 : r0 + P, :], in_=res[:])

                st = stats_pool.tile(
                    [P, HIDDEN // BN_CHUNK, 6], mybir.dt.float32, tag="st"
                )
                for j in range(HIDDEN // BN_CHUNK):
                    nc.vector.bn_stats(
                        st[:, j, :], res[:, j * BN_CHUNK : (j + 1) * BN_CHUNK]
                    )
                mv = stats_pool.tile([P, 2], mybir.dt.float32, tag="mv")
                nc.vector.bn_aggr(mv[:], st[:])

                # ms = mean(res^2) = var + mean^2
                m2 = stats_pool.tile([P, 1], mybir.dt.float32, tag="m2")
                nc.scalar.square(m2[:], mv[:, 0:1])
                ms = stats_pool.tile([P, 1], mybir.dt.float32, tag="ms")
                nc.vector.tensor_add(ms[:], m2[:], mv[:, 1:2])
                rms = stats_pool.tile([P, 1], mybir.dt.float32, tag="rms")
                nc.scalar.activation(
                    rms[:], ms[:], mybir.ActivationFunctionType.Sqrt,
                    bias=eps_t[:], scale=1.0,
                )
                rstd = stats_pool.tile([P, 1], mybir.dt.float32, tag="rstd")
                nc.vector.reciprocal(rstd[:], rms[:])

                # xf is dead after the residual add — reuse it for res_new * w
                nc.vector.tensor_mul(xf[:], res[:], w_b[:])
                o8 = work.tile([P, HIDDEN], mybir.dt.int8, tag="o8")
                nc.scalar.mul(o8[:], xf[:], rstd[:])
                nc.sync.dma_start(out=out_i8[r0 : r0 + P, :], in_=o8[:])

    nc.compile()
    return nc


def kernel(residual, x, weight, a):
    global last_results
    residual = np.ascontiguousarray(residual, dtype=np.float32)
    x = np.ascontiguousarray(x, dtype=np.int32)
    weight = np.ascontiguousarray(weight, dtype=np.float32)
    a_f = float(np.asarray(a))

    if a_f not in _cache:
        _cache[a_f] = _build(a_f)
    nc = _cache[a_f]

    in_maps = [
        {
            "residual": residual[c * ROWS : (c + 1) * ROWS],
            "x": x[c * ROWS : (c + 1) * ROWS],
            "weight": weight,
        }
        for c in range(N_CORES)
    ]
    trace = os.environ.get("BASS_KERNEL_TRACE") == "1"
    last_results = run_bass_kernel_spmd(
        nc, in_maps, list(range(N_CORES)), trace=trace
    )
    res = last_results.results
    res_new = np.concatenate([res[c]["res_new"] for c in range(N_CORES)], axis=0)
    out_i8 = np.concatenate([res[c]["out_i8"] for c in range(N_CORES)], axis=0)
    return res_new, out_i8


# revision 5
# speedup vs baseline: 1.0208x; 1.0208x over previous
"""Fused dequant + residual-add + RMSNorm + int8 requant for TRN2 (8 NeuronCores).

Sharding: tokens (rows) split evenly across the 8 cores; the hidden-dim
reduction stays local and `weight` is replicated.

Per core (2048 x 4096), per 128-row tile:
  ACT : xf = x_i32 * a                     (dequant, int32 -> f32)
  DVE : res_new = residual + xf            (tensor_tensor add, in place)
  DVE : bn_stats/bn_aggr -> mean, var      (mean(res^2) = var + mean^2)
  ACT : rms = sqrt(ms + eps);  DVE: rstd = 1/rms
  DVE : yw = res_new * w                   (weight broadcast over partitions)
  ACT : out_i8 = yw * rstd                 (f32 -> int8 is RNE + saturating)

The f32->int8 output conversion on ACT/DVE was verified on hardware to match
np.clip(np.round(x), -128, 127) exactly, so no explicit rounding pass is needed.
"""

import os

import numpy as np

import concourse.bacc as bacc
import concourse.bass as bass
import concourse.tile as tile
from concourse import mybir
from concourse.bass_utils import run_bass_kernel_spmd

TOKENS = 16384
HIDDEN = 4096
N_CORES = 8
ROWS = TOKENS // N_CORES  # 2048 rows per core
P = 128                   # SBUF partitions
NT = ROWS // P            # 16 row-tiles per core
EPS = 1e-6
BN_CHUNK = 512            # bn_stats free-dim hardware limit

_cache: dict = {}
last_results = None  # BassKernelResults of the most recent run (for profiling)


def _build(a: float):
    nc = bacc.Bacc(
        "TRN2", target_bir_lowering=False, debug=False, num_devices=N_CORES
    )
    residual = nc.dram_tensor(
        "residual", [ROWS, HIDDEN], mybir.dt.float32, kind="ExternalInput"
    ).ap()
    x = nc.dram_tensor(
        "x", [ROWS, HIDDEN], mybir.dt.int32, kind="ExternalInput"
    ).ap()
    weight = nc.dram_tensor(
        "weight", [HIDDEN], mybir.dt.float32, kind="ExternalInput"
    ).ap()
    res_new = nc.dram_tensor(
        "res_new", [ROWS, HIDDEN], mybir.dt.float32, kind="ExternalOutput"
    ).ap()
    out_i8 = nc.dram_tensor(
        "out_i8", [ROWS, HIDDEN], mybir.dt.int8, kind="ExternalOutput"
    ).ap()

    with tile.TileContext(nc) as tc:
        with (
            tc.tile_pool(name="singles", bufs=1) as singles,
            tc.tile_pool(name="work", bufs=4) as work,
            tc.tile_pool(name="stats", bufs=4) as stats_pool,
        ):
            # weight broadcast to all 128 partitions (DRAM sources allow a
            # 0-step partition AP; SBUF sources do not)
            w_b = singles.tile([P, HIDDEN], mybir.dt.float32)
            nc.gpsimd.dma_start(out=w_b[:], in_=weight[None, :].partition_broadcast(P))
            eps_t = singles.tile([P, 1], mybir.dt.float32)
            nc.vector.memset(eps_t[:], EPS)

            for it in range(NT):
                r0 = it * P
                # loads on the SP HWDGE ring; stores on gpsimd/SWDGE so a
                # stalled load never blocks a ready store (FIFO per ring)
                xi = work.tile([P, HIDDEN], mybir.dt.int32, tag="xi")
                nc.sync.dma_start(out=xi[:], in_=x[r0 : r0 + P, :])
                res = work.tile([P, HIDDEN], mybir.dt.float32, tag="res")
                nc.sync.dma_start(out=res[:], in_=residual[r0 : r0 + P, :])

                # dequant in place: reinterpret the int32 tile as f32
                xf = xi[:].bitcast(mybir.dt.float32)
                nc.scalar.mul(xf, xi[:], a)
                nc.vector.tensor_add(res[:], res[:], xf)
                nc.gpsimd.dma_start(out=res_new[r0 : r0 + P, :], in_=res[:])

                st = stats_pool.tile(
                    [P, HIDDEN // BN_CHUNK, 6], mybir.dt.float32, tag="st"
                )
                for j in range(HIDDEN // BN_CHUNK):
                    nc.vector.bn_stats(
                        st[:, j, :], res[:, j * BN_CHUNK : (j + 1) * BN_CHUNK]
                    )
                mv = stats_pool.tile([P, 2], mybir.dt.float32, tag="mv")
                nc.vector.bn_aggr(mv[:], st[:])

                # ms = mean(res^2) = var + mean^2
                m2 = stats_pool.tile([P, 1], mybir.dt.float32, tag="m2")
                nc.scalar.square(m2[:], mv[:, 0:1])
                ms = stats_pool.tile([P, 1], mybir.dt.float32, tag="ms")
                nc.vector.tensor_add(ms[:], m2[:], mv[:, 1:2])
                rms = stats_pool.tile([P, 1], mybir.dt.float32, tag="rms")
                nc.scalar.activation(
                    rms[:], ms[:], mybir.ActivationFunctionType.Sqrt,
                    bias=eps_t[:], scale=1.0,
                )
                rstd = stats_pool.tile([P, 1], mybir.dt.float32, tag="rstd")
                nc.vector.reciprocal(rstd[:], rms[:])

                # xf is dead after the residual add — reuse it for res_new * w
                nc.vector.tensor_mul(xf, res[:], w_b[:])
                o8 = work.tile([P, HIDDEN], mybir.dt.int8, tag="o8")
                nc.scalar.mul(o8[:], xf, rstd[:])
                nc.gpsimd.dma_start(out=out_i8[r0 : r0 + P, :], in_=o8[:])

    nc.compile()
    return nc


def kernel(residual, x, weight, a):
    global last_results
    residual = np.ascontiguousarray(residual, dtype=np.float32)
    x = np.ascontiguousarray(x, dtype=np.int32)
    weight = np.ascontiguousarray(weight, dtype=np.float32)
    a_f = float(np.asarray(a))

    if a_f not in _cache:
        _cache[a_f] = _build(a_f)
    nc = _cache[a_f]

    in_maps = [
        {
            "residual": residual[c * ROWS : (c + 1) * ROWS],
            "x": x[c * ROWS : (c + 1) * ROWS],
            "weight": weight,
        }
        for c in range(N_CORES)
    ]
    trace = os.environ.get("BASS_KERNEL_TRACE") == "1"
    last_results = run_bass_kernel_spmd(
        nc, in_maps, list(range(N_CORES)), trace=trace
    )
    res = last_results.results
    res_new = np.concatenate([res[c]["res_new"] for c in range(N_CORES)], axis=0)
    out_i8 = np.concatenate([res[c]["out_i8"] for c in range(N_CORES)], axis=0)
    return res_new, out_i8


# revision 6
# speedup vs baseline: 1.0228x; 1.0020x over previous
"""Fused dequant + residual-add + RMSNorm + int8 requant for TRN2 (8 NeuronCores).

Sharding: tokens (rows) split evenly across the 8 cores; the hidden-dim
reduction stays local and `weight` is replicated.

Per core (2048 x 4096), per 128-row tile:
  ACT : xf = x_i32 * a                       (dequant, int32 -> f32, in place)
  DVE : res_new = residual + xf              (tensor_tensor add, in place)
  ACT : Square(res_new) with accum_out       (ssq = sum(res_new^2) per row)
  ACT : rms = sqrt(ssq/H + eps);  DVE: rstd = 1/rms
  DVE : yw = res_new * w                     (weight broadcast over partitions)
  ACT : out_i8 = yw * rstd                   (f32 -> int8 is RNE + saturating)

Loads ride the SP HWDGE ring, stores ride gpsimd/SWDGE so a stalled load
never queues ahead of a ready store. The f32->int8 output conversion was
verified on hardware to match np.clip(np.round(x), -128, 127) exactly.
"""

import os

import numpy as np

import concourse.bacc as bacc
import concourse.bass as bass
import concourse.tile as tile
from concourse import mybir
from concourse.bass_utils import run_bass_kernel_spmd

TOKENS = 16384
HIDDEN = 4096
N_CORES = 8
ROWS = TOKENS // N_CORES  # 2048 rows per core
P = 128                   # SBUF partitions
NT = ROWS // P            # 16 row-tiles per core
EPS = 1e-6

_cache: dict = {}
last_results = None  # BassKernelResults of the most recent run (for profiling)


def _build(a: float):
    nc = bacc.Bacc(
        "TRN2", target_bir_lowering=False, debug=False, num_devices=N_CORES
    )
    residual = nc.dram_tensor(
        "residual", [ROWS, HIDDEN], mybir.dt.float32, kind="ExternalInput"
    ).ap()
    x = nc.dram_tensor(
        "x", [ROWS, HIDDEN], mybir.dt.int32, kind="ExternalInput"
    ).ap()
    weight = nc.dram_tensor(
        "weight", [HIDDEN], mybir.dt.float32, kind="ExternalInput"
    ).ap()
    res_new = nc.dram_tensor(
        "res_new", [ROWS, HIDDEN], mybir.dt.float32, kind="ExternalOutput"
    ).ap()
    out_i8 = nc.dram_tensor(
        "out_i8", [ROWS, HIDDEN], mybir.dt.int8, kind="ExternalOutput"
    ).ap()

    with tile.TileContext(nc) as tc:
        with (
            tc.tile_pool(name="singles", bufs=1) as singles,
            tc.tile_pool(name="work", bufs=4) as work,
            tc.tile_pool(name="sq", bufs=1) as sq_pool,
            tc.tile_pool(name="stats", bufs=4) as stats_pool,
        ):
            # weight: one 16 KiB HBM read into partition 0, then log2-double
            # across partitions with SBUF->SBUF DMAs (keeps 2 MiB off HBM)
            w_b = singles.tile([P, HIDDEN], mybir.dt.float32)
            nc.sync.dma_start(out=w_b[0:1, :], in_=weight[None, :])
            p = 1
            while p < P:
                n = min(p, P - p)
                nc.gpsimd.dma_start(out=w_b[p : p + n, :], in_=w_b[0:n, :])
                p += n
            eps_t = singles.tile([P, 1], mybir.dt.float32)
            nc.vector.memset(eps_t[:], EPS)
            # scratch for Square's elementwise output (only accum_out is used;
            # ACT executes in order so one buffer never races itself)
            sq = sq_pool.tile([P, HIDDEN], mybir.dt.float32)

            for it in range(NT):
                r0 = it * P
                xi = work.tile([P, HIDDEN], mybir.dt.int32, tag="xi")
                nc.sync.dma_start(out=xi[:], in_=x[r0 : r0 + P, :])
                res = work.tile([P, HIDDEN], mybir.dt.float32, tag="res")
                nc.sync.dma_start(out=res[:], in_=residual[r0 : r0 + P, :])

                # dequant in place: reinterpret the int32 tile as f32
                xf = xi[:].bitcast(mybir.dt.float32)
                nc.scalar.mul(xf, xi[:], a)
                nc.vector.tensor_add(res[:], res[:], xf)
                nc.gpsimd.dma_start(out=res_new[r0 : r0 + P, :], in_=res[:])

                # ssq = sum(res^2) along hidden; rms = sqrt(ssq/H + eps)
                ssq = stats_pool.tile([P, 1], mybir.dt.float32, tag="ssq")
                nc.scalar.activation(
                    sq[:], res[:], mybir.ActivationFunctionType.Square,
                    accum_out=ssq[:],
                )
                rms = stats_pool.tile([P, 1], mybir.dt.float32, tag="rms")
                nc.scalar.activation(
                    rms[:], ssq[:], mybir.ActivationFunctionType.Sqrt,
                    bias=eps_t[:], scale=1.0 / HIDDEN,
                )
                rstd = stats_pool.tile([P, 1], mybir.dt.float32, tag="rstd")
                nc.vector.reciprocal(rstd[:], rms[:])

                # xf is dead after the residual add — reuse it for res_new * w
                nc.vector.tensor_mul(xf, res[:], w_b[:])
                o8 = work.tile([P, HIDDEN], mybir.dt.int8, tag="o8")
                nc.scalar.mul(o8[:], xf, rstd[:])
                nc.gpsimd.dma_start(out=out_i8[r0 : r0 + P, :], in_=o8[:])

    nc.compile()
    return nc


def kernel(residual, x, weight, a):
    global last_results
    residual = np.ascontiguousarray(residual, dtype=np.float32)
    x = np.ascontiguousarray(x, dtype=np.int32)
    weight = np.ascontiguousarray(weight, dtype=np.float32)
    a_f = float(np.asarray(a))

    if a_f not in _cache:
        _cache[a_f] = _build(a_f)
    nc = _cache[a_f]

    in_maps = [
        {
            "residual": residual[c * ROWS : (c + 1) * ROWS],
            "x": x[c * ROWS : (c + 1) * ROWS],
            "weight": weight,
        }
        for c in range(N_CORES)
    ]
    trace = os.environ.get("BASS_KERNEL_TRACE") == "1"
    last_results = run_bass_kernel_spmd(
        nc, in_maps, list(range(N_CORES)), trace=trace
    )
    res = last_results.results
    res_new = np.concatenate([res[c]["res_new"] for c in range(N_CORES)], axis=0)
    out_i8 = np.concatenate([res[c]["out_i8"] for c in range(N_CORES)], axis=0)
    return res_new, out_i8


# revision 7
# speedup vs baseline: 1.2848x; 1.2561x over previous
"""Fused dequant + residual-add + RMSNorm + int8 requant for TRN2 (8 NeuronCores).

Sharding: tokens (rows) split evenly across the 8 cores; the hidden-dim
reduction stays local and `weight` is replicated.

The quantized GEMM output `x` arrives as int32 but its values fit int16, so
the host narrows it before upload — that cuts the dominant HBM read stream
in half (int16 -> f32 -> *a is bit-identical to int32 -> f32 -> *a for
|x| < 2^24). A range check falls back to the exact int32 kernel.

Per core (2048 x 4096), per 128-row tile:
  ACT : xf = x * a                           (dequant -> f32, in place)
  DVE : res_new = residual + xf              (tensor_tensor add, in place)
  ACT : Square(res_new) with accum_out       (ssq = sum(res_new^2) per row)
  ACT : rms = sqrt(ssq/H + eps);  DVE: rstd = 1/rms
  DVE : yw = res_new * w                     (weight broadcast over partitions)
  ACT : out_i8 = yw * rstd                   (f32 -> int8 is RNE + saturating)

Loads ride the SP HWDGE ring, stores ride gpsimd/SWDGE so a stalled load
never queues ahead of a ready store. The f32->int8 output conversion was
verified on hardware to match np.clip(np.round(x), -128, 127) exactly.

The int16 dequant reads the packed int16 from the TOP half of the f32 tile
and writes f32 over the whole tile in place: the write pointer (4 B/elem
from offset 0) stays strictly behind the read pointer (2 B/elem from half
way), so a streaming elementwise engine never clobbers unread input.
"""

import os

import numpy as np

import concourse.bacc as bacc
import concourse.bass as bass
import concourse.tile as tile
from concourse import mybir
from concourse.bass_utils import run_bass_kernel_spmd

TOKENS = 16384
HIDDEN = 4096
N_CORES = 8
ROWS = TOKENS // N_CORES  # 2048 rows per core
P = 128                   # SBUF partitions
NT = ROWS // P            # 16 row-tiles per core
EPS = 1e-6

_cache: dict = {}
last_results = None  # BassKernelResults of the most recent run (for profiling)


def _build(a: float, x_dtype):
    nc = bacc.Bacc(
        "TRN2", target_bir_lowering=False, debug=False, num_devices=N_CORES
    )
    residual = nc.dram_tensor(
        "residual", [ROWS, HIDDEN], mybir.dt.float32, kind="ExternalInput"
    ).ap()
    x = nc.dram_tensor("x", [ROWS, HIDDEN], x_dtype, kind="ExternalInput").ap()
    weight = nc.dram_tensor(
        "weight", [HIDDEN], mybir.dt.float32, kind="ExternalInput"
    ).ap()
    res_new = nc.dram_tensor(
        "res_new", [ROWS, HIDDEN], mybir.dt.float32, kind="ExternalOutput"
    ).ap()
    out_i8 = nc.dram_tensor(
        "out_i8", [ROWS, HIDDEN], mybir.dt.int8, kind="ExternalOutput"
    ).ap()

    with tile.TileContext(nc) as tc:
        with (
            tc.tile_pool(name="singles", bufs=1) as singles,
            tc.tile_pool(name="work", bufs=4) as work,
            tc.tile_pool(name="sq", bufs=1) as sq_pool,
            tc.tile_pool(name="stats", bufs=4) as stats_pool,
        ):
            # weight: one 16 KiB HBM read into partition 0, then log2-double
            # across partitions with SBUF->SBUF DMAs (keeps 2 MiB off HBM)
            w_b = singles.tile([P, HIDDEN], mybir.dt.float32)
            nc.sync.dma_start(out=w_b[0:1, :], in_=weight[None, :])
            p = 1
            while p < P:
                n = min(p, P - p)
                nc.gpsimd.dma_start(out=w_b[p : p + n, :], in_=w_b[0:n, :])
                p += n
            eps_t = singles.tile([P, 1], mybir.dt.float32)
            nc.vector.memset(eps_t[:], EPS)
            # scratch for Square's elementwise output (only accum_out is used;
            # ACT executes in order so one buffer never races itself)
            sq = sq_pool.tile([P, HIDDEN], mybir.dt.float32)

            for it in range(NT):
                r0 = it * P
                xi = work.tile([P, HIDDEN], mybir.dt.float32, tag="xi")
                xf = xi[:]
                if x_dtype == mybir.dt.int16:
                    xi_in = xi[:].bitcast(mybir.dt.int16)[:, HIDDEN : 2 * HIDDEN]
                else:
                    xi_in = xi[:].bitcast(mybir.dt.int32)
                nc.sync.dma_start(out=xi_in, in_=x[r0 : r0 + P, :])
                res = work.tile([P, HIDDEN], mybir.dt.float32, tag="res")
                nc.sync.dma_start(out=res[:], in_=residual[r0 : r0 + P, :])

                nc.scalar.mul(xf, xi_in, a)  # dequant in place
                nc.vector.tensor_add(res[:], res[:], xf)
                nc.gpsimd.dma_start(out=res_new[r0 : r0 + P, :], in_=res[:])

                # ssq = sum(res^2) along hidden; rms = sqrt(ssq/H + eps)
                ssq = stats_pool.tile([P, 1], mybir.dt.float32, tag="ssq")
                nc.scalar.activation(
                    sq[:], res[:], mybir.ActivationFunctionType.Square,
                    accum_out=ssq[:],
                )
                rms = stats_pool.tile([P, 1], mybir.dt.float32, tag="rms")
                nc.scalar.activation(
                    rms[:], ssq[:], mybir.ActivationFunctionType.Sqrt,
                    bias=eps_t[:], scale=1.0 / HIDDEN,
                )
                rstd = stats_pool.tile([P, 1], mybir.dt.float32, tag="rstd")
                nc.vector.reciprocal(rstd[:], rms[:])

                # xi/xf is dead after the residual add — reuse for res_new * w
                nc.vector.tensor_mul(xf, res[:], w_b[:])
                o8 = work.tile([P, HIDDEN], mybir.dt.int8, tag="o8")
                nc.scalar.mul(o8[:], xf, rstd[:])
                nc.gpsimd.dma_start(out=out_i8[r0 : r0 + P, :], in_=o8[:])

    nc.compile()
    return nc


def kernel(residual, x, weight, a):
    global last_results
    residual = np.ascontiguousarray(residual, dtype=np.float32)
    x = np.ascontiguousarray(x, dtype=np.int32)
    weight = np.ascontiguousarray(weight, dtype=np.float32)
    a_f = float(np.asarray(a))

    if x.min() >= -32768 and x.max() <= 32767:
        x_send = x.astype(np.int16)
        key = (a_f, "i16")
        x_dtype = mybir.dt.int16
    else:
        x_send = x
        key = (a_f, "i32")
        x_dtype = mybir.dt.int32

    if key not in _cache:
        _cache[key] = _build(a_f, x_dtype)
    nc = _cache[key]

    in_maps = [
        {
            "residual": residual[c * ROWS : (c + 1) * ROWS],
            "x": x_send[c * ROWS : (c + 1) * ROWS],
            "weight": weight,
        }
        for c in range(N_CORES)
    ]
    trace = os.environ.get("BASS_KERNEL_TRACE") == "1"
    last_results = run_bass_kernel_spmd(
        nc, in_maps, list(range(N_CORES)), trace=trace
    )
    res = last_results.results
    res_new = np.concatenate([res[c]["res_new"] for c in range(N_CORES)], axis=0)
    out_i8 = np.concatenate([res[c]["out_i8"] for c in range(N_CORES)], axis=0)
    return res_new, out_i8


# revision 9
# speedup vs baseline: 1.2929x; 1.0063x over previous
"""Fused dequant + residual-add + RMSNorm + int8 requant for TRN2 (8 NeuronCores).

Sharding: tokens (rows) split evenly across the 8 cores; the hidden-dim
reduction stays local and `weight` is replicated.

The quantized GEMM output `x` arrives as int32 but its values fit int16, so
the host narrows it before upload — that cuts the dominant HBM read stream
in half (int16 -> f32 -> *a is bit-identical to int32 -> f32 -> *a for
|x| < 2^24). A range check falls back to the exact int32 kernel.

Per core (2048 x 4096), per 128-row tile:
  ACT : xf = x * a                           (dequant -> f32, in place)
  DVE : res_new = residual + xf              (tensor_tensor add, in place)
  ACT : Square(res_new) with accum_out       (ssq = sum(res_new^2) per row)
  ACT : rms = sqrt(ssq/H + eps);  DVE: rstd = 1/rms
  DVE : yw = res_new * w                     (weight broadcast over partitions)
  ACT : out_i8 = yw * rstd                   (f32 -> int8 is RNE + saturating)

Loads ride the SP HWDGE ring, stores ride gpsimd/SWDGE so a stalled load
never queues ahead of a ready store. The f32->int8 output conversion was
verified on hardware to match np.clip(np.round(x), -128, 127) exactly.

The int16 dequant reads the packed int16 from the TOP half of the f32 tile
and writes f32 over the whole tile in place: the write pointer (4 B/elem
from offset 0) stays strictly behind the read pointer (2 B/elem from half
way), so a streaming elementwise engine never clobbers unread input.
"""

import os

import numpy as np

import concourse.bacc as bacc
import concourse.bass as bass
import concourse.tile as tile
from concourse import mybir
from concourse.bass_utils import run_bass_kernel_spmd

TOKENS = 16384
HIDDEN = 4096
N_CORES = 8
ROWS = TOKENS // N_CORES  # 2048 rows per core
P = 128                   # SBUF partitions
NT = ROWS // P            # 16 row-tiles per core
EPS = 1e-6
SPLIT = 2560              # final-requant column split: ACT [0:SPLIT], DVE [SPLIT:]

_cache: dict = {}
last_results = None  # BassKernelResults of the most recent run (for profiling)


def _build(a: float, x_dtype):
    nc = bacc.Bacc(
        "TRN2", target_bir_lowering=False, debug=False, num_devices=N_CORES
    )
    residual = nc.dram_tensor(
        "residual", [ROWS, HIDDEN], mybir.dt.float32, kind="ExternalInput"
    ).ap()
    x = nc.dram_tensor("x", [ROWS, HIDDEN], x_dtype, kind="ExternalInput").ap()
    weight = nc.dram_tensor(
        "weight", [HIDDEN], mybir.dt.float32, kind="ExternalInput"
    ).ap()
    res_new = nc.dram_tensor(
        "res_new", [ROWS, HIDDEN], mybir.dt.float32, kind="ExternalOutput"
    ).ap()
    out_i8 = nc.dram_tensor(
        "out_i8", [ROWS, HIDDEN], mybir.dt.int8, kind="ExternalOutput"
    ).ap()

    with tile.TileContext(nc) as tc:
        with (
            tc.tile_pool(name="singles", bufs=1) as singles,
            tc.tile_pool(name="work", bufs=4) as work,
            tc.tile_pool(name="sq", bufs=1) as sq_pool,
            tc.tile_pool(name="stats", bufs=4) as stats_pool,
        ):
            # weight: one 16 KiB HBM read into partition 0, then log2-double
            # across partitions with SBUF->SBUF DMAs (keeps 2 MiB off HBM)
            w_b = singles.tile([P, HIDDEN], mybir.dt.float32)
            nc.sync.dma_start(out=w_b[0:1, :], in_=weight[None, :])
            p = 1
            while p < P:
                n = min(p, P - p)
                nc.gpsimd.dma_start(out=w_b[p : p + n, :], in_=w_b[0:n, :])
                p += n
            eps_t = singles.tile([P, 1], mybir.dt.float32)
            nc.vector.memset(eps_t[:], EPS)
            # scratch for Square's elementwise output (only accum_out is used;
            # ACT executes in order so one buffer never races itself)
            sq = sq_pool.tile([P, HIDDEN], mybir.dt.float32)

            for it in range(NT):
                r0 = it * P
                xi = work.tile([P, HIDDEN], mybir.dt.float32, tag="xi")
                xf = xi[:]
                if x_dtype == mybir.dt.int16:
                    xi_in = xi[:].bitcast(mybir.dt.int16)[:, HIDDEN : 2 * HIDDEN]
                else:
                    xi_in = xi[:].bitcast(mybir.dt.int32)
                nc.sync.dma_start(out=xi_in, in_=x[r0 : r0 + P, :])
                res = work.tile([P, HIDDEN], mybir.dt.float32, tag="res")
                nc.sync.dma_start(out=res[:], in_=residual[r0 : r0 + P, :])

                nc.scalar.mul(xf, xi_in, a)  # dequant in place
                nc.vector.tensor_add(res[:], res[:], xf)
                nc.gpsimd.dma_start(out=res_new[r0 : r0 + P, :], in_=res[:])

                # ssq = sum(res^2) along hidden; rms = sqrt(ssq/H + eps)
                ssq = stats_pool.tile([P, 1], mybir.dt.float32, tag="ssq")
                nc.scalar.activation(
                    sq[:], res[:], mybir.ActivationFunctionType.Square,
                    accum_out=ssq[:],
                )
                rms = stats_pool.tile([P, 1], mybir.dt.float32, tag="rms")
                nc.scalar.activation(
                    rms[:], ssq[:], mybir.ActivationFunctionType.Sqrt,
                    bias=eps_t[:], scale=1.0 / HIDDEN,
                )
                rstd = stats_pool.tile([P, 1], mybir.dt.float32, tag="rstd")
                nc.vector.reciprocal(rstd[:], rms[:])

                # xi/xf is dead after the residual add — reuse for res_new * w
                nc.vector.tensor_mul(xf, res[:], w_b[:])
                # final requant split across ACT and DVE to balance busy time
                # (both convert f32 -> int8 with identical RNE + saturation)
                o8 = work.tile([P, HIDDEN], mybir.dt.int8, tag="o8")
                nc.scalar.mul(o8[:, :SPLIT], xf[:, :SPLIT], rstd[:])
                nc.vector.tensor_scalar_mul(o8[:, SPLIT:], xf[:, SPLIT:], rstd[:])
                nc.gpsimd.dma_start(out=out_i8[r0 : r0 + P, :], in_=o8[:])

    nc.compile()
    return nc


def kernel(residual, x, weight, a):
    global last_results
    residual = np.ascontiguousarray(residual, dtype=np.float32)
    x = np.ascontiguousarray(x, dtype=np.int32)
    weight = np.ascontiguousarray(weight, dtype=np.float32)
    a_f = float(np.asarray(a))

    if x.min() >= -32768 and x.max() <= 32767:
        x_send = x.astype(np.int16)
        key = (a_f, "i16")
        x_dtype = mybir.dt.int16
    else:
        x_send = x
        key = (a_f, "i32")
        x_dtype = mybir.dt.int32

    if key not in _cache:
        _cache[key] = _build(a_f, x_dtype)
    nc = _cache[key]

    in_maps = [
        {
            "residual": residual[c * ROWS : (c + 1) * ROWS],
            "x": x_send[c * ROWS : (c + 1) * ROWS],
            "weight": weight,
        }
        for c in range(N_CORES)
    ]
    trace = os.environ.get("BASS_KERNEL_TRACE") == "1"
    last_results = run_bass_kernel_spmd(
        nc, in_maps, list(range(N_CORES)), trace=trace
    )
    res = last_results.results
    res_new = np.concatenate([res[c]["res_new"] for c in range(N_CORES)], axis=0)
    out_i8 = np.concatenate([res[c]["out_i8"] for c in range(N_CORES)], axis=0)
    return res_new, out_i8
